# revision 4
# baseline (speedup 1.0000x reference)
"""DeepFourierTransform kernel for Trainium2 (8 NeuronCores, data-parallel).

Problem:
  x [4096, 4096] f32 -> sliding windows (31 per row, size 256, hop 128)
  cos_feat = cos(win @ w_cos.T + b_cos)   [B, 31, 512]
  sin_feat = sin(win @ w_sin.T + b_sin)   [B, 31, 512]
  out = concat(cos,sin) @ w_out.T + b_out, mean over windows, log_softmax
  -> [4096, 4] f32

Strategy (per core, batch shard of 512 rows):
  - Main matmuls in fp8-e4m3 DoubleRow perf mode: K=256 (one window) as 2
    k-tiles of 128 in ONE matmul at 0.5 cycles/row -> PE main cost ~27us
    (vs ~107us bf16).  fp8 quantization noise washes out over the
    31x1024-feature mean (validated: end-to-end L2 ~1.4e-3 vs 2e-2 gate).
  - Window accumulation: instead of DVE tree-adds (~77us), every window's
    feature tile is immediately projected onto the 4 outputs by tiny PE
    matmuls (lhsT = 128x128 feat block, rhs = w_out.T/31 [128,4], out
    [128,4]) accumulating into one persistent PSUM bank (memset once,
    start=False).  992 matmuls x ~1.7ns engine time.
  - The 16.25M Sin/Cos evals split across two engines:
      * ACT: Sin with fused per-partition bias, 0.833ns/elem.
      * DVE: degree-6 factored polynomial for cos combos,
        cos(v) ~ cc*(u - r1)*((u - Re)^2 + Im2), u = v^2:
        1 TT mult @1x (PSUM f32 read) + 3 tensor_scalar @4x + 2 TT @2x
        ~ 2.9ns/elem.  Bias is pre-added into PSUM by a K=1 bf16 matmul
        (b_cos row x ones) so the poly runs on biased z directly.
  - PSUM: tile A [128,4,512] (4 banks) + tile B [128,3,512] (3 banks)
    ping-pong at chunk granularity + 1 bank for the output accumulator.
    Slot order: lone 3-window block first (combos alternate A/B), then 4
    pairs of (4w on A, 3w on B) per combo, cos/sin interleaved so the 16
    DVE chunks (pair-B, cos combos) are spaced 4 slots apart.
  - Projections for slot s are emitted after mains of slot s+2 so PE never
    head-of-line blocks on a feat tile.
  - Tail: z = pf + b_out, batched log_softmax (no max-shift: |z|<=~3).
  - Exp/Ln steered to the shared natural_log_exp table set; a warmup Sin
    pulls the trig table load to t~0; dummy matmuls pre-warm the PE clock.
"""

import numpy as np
import ml_dtypes

import concourse.bass as bass
import concourse.bacc as bacc
import concourse.mybir as mybir
import concourse.tile as tile
from concourse.bass_utils import run_bass_kernel_spmd

BF16 = mybir.dt.bfloat16
F32 = mybir.dt.float32
FP8 = mybir.dt.float8e4

N_CORES = 8
B = 4096
B_LOCAL = B // N_CORES          # 512
SEQ = 4096
P = 128
NCHUNK = SEQ // P               # 32
NWIN = 31
M = 512                         # features per trig branch
NCOMBO = 8                      # 4 cos m-tiles + 4 sin m-tiles
OUT_DIM = 4
NBT = B_LOCAL // P              # 4 batch tiles of 128

# degree-6 (in v) minimax-ish polynomial for cos(v), |v| <= 3.85:
#   cos(v) ~ CC*(u - R1)*((u - RE)^2 + IM2),  u = v^2
# max err 6.1e-3 on the range; fitted offline.
CC = -0.0008059618890211334
R1 = 2.4857771759454126
RE = 21.54723134529601
IM2 = 31.799023222338658

_CACHED_NC = None
NWARM = 6  # PE/HAM warmup matmuls issued during the initial DMA wait
DR = mybir.MatmulPerfMode.DoubleRow


def _make_slots():
    """Chunk schedule: (tile 'A'|'B', w0, nw, combo, engine 'ACT'|'DVE').

    Consecutive slots alternate PSUM tiles A/B (ping-pong).  The lone
    3-window block (w28-30) runs first (its x chunks are DMA'd first);
    pairs of (4w, 3w) follow, combos ordered cos/sin interleaved so DVE
    (pair-B cos chunks) gets work every 4th slot.
    """
    slots = []
    for i, c in enumerate([4, 0, 5, 1, 6, 2, 7, 3]):
        slots.append(("A" if i % 2 == 0 else "B", 28, 3, c, "ACT"))
    for p in range(4):
        for c in [0, 4, 1, 5, 2, 6, 3, 7]:
            slots.append(("A", 7 * p, 4, c, "ACT"))
            slots.append(
                ("B", 7 * p + 4, 3, c, "DVE" if c < 4 else "ACT")
            )
    assert sum(nw for t, w0, nw, c, e in slots) == NWIN * NCOMBO
    return slots


class _Bacc(bacc.Bacc):
    """Bacc with a curated activation-table list: Exp/Ln resolve to the shared
    natural_log_exp_and_others set (one tail table load instead of two)."""

    def insert_act_table_loads(self):
        import bass_rust as _br
        from concourse.hw_specs import get_activation_tables

        has_activation = any(
            isinstance(i, mybir.InstActivation)
            for b in self.main_func.blocks
            for i in b.instructions
        )
        if not has_activation:
            return
        act = mybir.ActivationFunctionType
        tables = list(get_activation_tables(self.m.arch).items())
        names = [n for n, _ in tables]
        if "natural_log_exp_and_others" in names:
            keep = names.index("natural_log_exp_and_others")
            tables = [
                (
                    n,
                    fns
                    if i == keep
                    else {f for f in fns if f not in (act.Exp, act.Ln)},
                )
                for i, (n, fns) in enumerate(tables)
            ]
        _br.insert_act_table_loads(self, tables)


def _build_nc():
    nc = _Bacc()
    act = mybir.ActivationFunctionType
    alu = mybir.AluOpType

    x = nc.dram_tensor("x", [SEQ, B_LOCAL], FP8, kind="ExternalInput")  # xT
    wt = nc.dram_tensor("wt", [P, NCOMBO, 2, P], FP8, kind="ExternalInput")
    bias = nc.dram_tensor("bias", [P, NCOMBO], F32, kind="ExternalInput")
    biasd = nc.dram_tensor("biasd", [1, 4, P], BF16, kind="ExternalInput")
    wot = nc.dram_tensor("wot", [P, NCOMBO, OUT_DIM], BF16, kind="ExternalInput")
    bot = nc.dram_tensor("bot", [P, OUT_DIM], F32, kind="ExternalInput")
    y = nc.dram_tensor("y", [B_LOCAL, OUT_DIM], F32, kind="ExternalOutput")

    slots = _make_slots()

    with tile.TileContext(nc) as tc:
        with (
            tc.tile_pool(name="consts", bufs=1) as consts,
            tc.tile_pool(name="xt", bufs=1) as xtp,
            tc.tile_pool(name="ftA", bufs=4) as ftAp,
            tc.tile_pool(name="ftB", bufs=4) as ftBp,
            tc.tile_pool(name="dvv", bufs=2) as dvv,
            tc.tile_pool(name="dvu", bufs=2) as dvu,
            tc.tile_pool(name="dvf1", bufs=2) as dvf1,
            tc.tile_pool(name="dvt", bufs=2) as dvt,
            tc.tile_pool(name="dvt2", bufs=2) as dvt2,
            tc.tile_pool(name="dvf2", bufs=2) as dvf2,
            tc.tile_pool(name="tail", bufs=2) as tailp,
        ):
            # ---- warmup: pull the Sin table load to t~0 on ACT ----
            warm = consts.tile([P, 1], F32)
            nc.vector.memset(warm, 0.0)
            warm2 = consts.tile([P, 1], F32)
            nc.scalar.activation(warm2, warm, act.Sin, scale=1.0)
            # PE warmup operand
            wrm = consts.tile([P, B_LOCAL], BF16)
            nc.vector.memset(wrm, 0.0)
            # ones row for the K=1 bias matmuls
            ones = consts.tile([1, B_LOCAL], BF16)
            nc.vector.memset(ones, 1.0)

            # ---- constants ----
            wt_sb = consts.tile([P, NCOMBO, 2, P], FP8)
            nc.sync.dma_start(wt_sb[:, 4:5], wt[:, 4:5])  # first lone combo
            nc.sync.dma_start(wt_sb[:, 0:4], wt[:, 0:4])
            nc.sync.dma_start(wt_sb[:, 5:], wt[:, 5:])
            bias_sb = consts.tile([P, NCOMBO], F32)
            nc.gpsimd.dma_start(bias_sb, bias[:, :])
            biasd_sb = consts.tile([1, 4, P], BF16)
            nc.gpsimd.dma_start(biasd_sb, biasd[:, :, :])
            wot_sb = consts.tile([P, NCOMBO, OUT_DIM], BF16)
            nc.gpsimd.dma_start(wot_sb, wot[:, :, :])
            bot_sb = consts.tile([P, OUT_DIM], F32)
            nc.gpsimd.dma_start(bot_sb, bot[:, :])

            # ---- x (transposed fp8 on host): lone-block chunks 28-31 first ----
            xt = xtp.tile([P, NCHUNK, B_LOCAL], FP8)
            GROUPS = [(28, 4), (0, 2), (2, 2), (4, 4), (8, 4), (12, 4),
                      (16, 4), (20, 4), (24, 4)]
            assert sum(g for _, g in GROUPS) == NCHUNK
            for k0, gsz in GROUPS:
                nc.sync.dma_start(
                    xt[:, k0 : k0 + gsz, :],
                    x[k0 * P : (k0 + gsz) * P, :].rearrange(
                        "(k p) b -> p k b", p=P
                    ),
                )

            with (
                tc.tile_pool(name="psA", bufs=1, space="PSUM") as psAp,
                tc.tile_pool(name="psB", bufs=1, space="PSUM") as psBp,
                tc.tile_pool(name="fft", bufs=1, space="PSUM") as fftp,
            ):
                psA = psAp.tile([P, 4, B_LOCAL], F32, tag="A")
                psB = psBp.tile([P, 3, B_LOCAL], F32, tag="B")
                fftb = fftp.tile([P, 512], F32, tag="fft")
                # zero the projection accumulator region (projections use
                # start=False so sub-bank groups never re-mark the bank's
                # pending-zero region)
                nc.vector.memset(fftb[:, : NBT * OUT_DIM], 0.0)

                if NWARM:
                    for _ in range(NWARM):
                        nc.tensor.matmul(
                            psA[0:1, 0, :],
                            lhsT=wrm[:, 0:1],
                            rhs=wrm,
                            start=True,
                            stop=True,
                        )

                def emit_mains(slot):
                    tname, w0, nw, c, eng = slot
                    ps = psA if tname == "A" else psB
                    for wi in range(nw):
                        w = w0 + wi
                        if eng == "DVE":
                            # pre-add bias via K=1 bf16 matmul (b_cos row)
                            nc.tensor.matmul(
                                ps[:, wi, :],
                                lhsT=biasd_sb[0:1, c, :],
                                rhs=ones[0:1, :],
                                start=True,
                                stop=False,
                                skip_group_check=True,
                            )
                        nc.tensor.matmul(
                            ps[:, wi, :],
                            lhsT=wt_sb[:, c, :, :],
                            rhs=xt[:, w : w + 2, :],
                            start=(eng != "DVE"),
                            stop=True,
                            perf_mode=DR,
                            skip_group_check=True,
                        )
                    return ps

                def emit_consumer(slot, ps):
                    tname, w0, nw, c, eng = slot
                    if eng == "ACT":
                        pool = ftAp if tname == "A" else ftBp
                        ft = pool.tile(
                            [P, 4 if tname == "A" else 3, B_LOCAL],
                            BF16,
                            tag="ft",
                        )
                        nc.scalar.activation(
                            ft[:, :nw, :],
                            ps[:, :nw, :],
                            act.Sin,
                            bias=bias_sb[:, c : c + 1],
                            scale=1.0,
                        )
                        return ft
                    # DVE polynomial: cos(v) = CC*(u-R1)*((u-RE)^2+IM2)
                    # (walrus rejects TT with two reads of the same PSUM AP,
                    # so copy v to bf16 SBUF first, then square there)
                    v = dvv.tile([P, 3, B_LOCAL], BF16, tag="v")
                    nc.vector.tensor_copy(v[:, :nw, :], ps[:, :nw, :])
                    vv = v[:, :nw, :]
                    u = dvu.tile([P, 3, B_LOCAL], BF16, tag="u")
                    nc.vector.tensor_tensor(u[:, :nw, :], vv, vv, alu.mult)
                    uv = u[:, :nw, :]
                    f1 = dvf1.tile([P, 3, B_LOCAL], BF16, tag="f1")
                    nc.vector.tensor_scalar(
                        f1[:, :nw, :], uv, CC, -CC * R1, alu.mult, alu.add
                    )
                    t = dvt.tile([P, 3, B_LOCAL], BF16, tag="t")
                    nc.vector.tensor_scalar_sub(t[:, :nw, :], uv, RE)
                    t2 = dvt2.tile([P, 3, B_LOCAL], BF16, tag="t2")
                    nc.vector.tensor_tensor(
                        t2[:, :nw, :], t[:, :nw, :], t[:, :nw, :], alu.mult
                    )
                    f2 = dvf2.tile([P, 3, B_LOCAL], BF16, tag="f2")
                    nc.vector.tensor_scalar_add(f2[:, :nw, :], t2[:, :nw, :], IM2)
                    ft = ftBp.tile([P, 3, B_LOCAL], BF16, tag="ft")
                    nc.vector.tensor_tensor(
                        ft[:, :nw, :], f1[:, :nw, :], f2[:, :nw, :], alu.mult
                    )
                    return ft

                def emit_proj(slot, ft, last):
                    tname, w0, nw, c, eng = slot
                    for wi in range(nw):
                        for bt in range(NBT):
                            nc.tensor.matmul(
                                fftb[:, bt * OUT_DIM : (bt + 1) * OUT_DIM],
                                lhsT=ft[:, wi, bt * P : (bt + 1) * P],
                                rhs=wot_sb[:, c, :],
                                start=False,
                                stop=(last and wi == nw - 1),
                                skip_group_check=True,
                            )

                pending = []
                for slot in slots:
                    ps = emit_mains(slot)
                    ft = emit_consumer(slot, ps)
                    pending.append((slot, ft))
                    if len(pending) > 2:
                        emit_proj(*pending.pop(0), last=False)
                for i, (slot, ft) in enumerate(pending):
                    emit_proj(slot, ft, last=(i == len(pending) - 1))

                # ---- tail: z = pf + b_out, batched log_softmax ----
                z_all = tailp.tile([P, NBT, OUT_DIM], F32, tag="z")
                for bt in range(NBT):
                    nc.vector.tensor_add(
                        z_all[:, bt, :],
                        fftb[:, bt * OUT_DIM : (bt + 1) * OUT_DIM],
                        bot_sb,
                    )
            e = tailp.tile([P, NBT, OUT_DIM], F32, tag="e")
            nc.scalar.activation(e, z_all, act.Exp)
            ssum = tailp.tile([P, NBT], F32, tag="ss")
            nc.vector.reduce_sum(ssum, e, axis=mybir.AxisListType.X)
            ls = tailp.tile([P, NBT], F32, tag="ls")
            nc.scalar.activation(ls, ssum, act.Ln)
            o = tailp.tile([P, NBT, OUT_DIM], F32, tag="o")
            nc.vector.tensor_tensor(
                o,
                z_all,
                ls[:, :, None].to_broadcast([P, NBT, OUT_DIM]),
                mybir.AluOpType.subtract,
            )
            nc.sync.dma_start(y.rearrange("(bt p) o -> p bt o", p=P), o)

    if not nc.is_finalized():
        nc.finalize()
    return nc


def _get_nc():
    global _CACHED_NC
    if _CACHED_NC is None:
        _CACHED_NC = _build_nc()
    return _CACHED_NC


def _make_in_maps(x, w_cos, b_cos, w_sin, b_sin, w_out, b_out):
    bf = ml_dtypes.bfloat16
    f8 = ml_dtypes.float8_e4m3
    x = np.asarray(x)
    w_cos, w_sin = np.asarray(w_cos), np.asarray(w_sin)
    b_cos, b_sin = np.asarray(b_cos), np.asarray(b_sin)
    w_out, b_out = np.asarray(w_out), np.asarray(b_out)
    # weights: [p, combo, ktile, m] fp8 (wt[p,c,j,m] = W[c*128+m, j*128+p])
    wt = np.concatenate([w_cos.T, w_sin.T], axis=1).reshape(2, P, NCOMBO, P)
    wt = np.ascontiguousarray(wt.transpose(1, 2, 0, 3)).astype(f8)
    # ACT bias: per-combo per-partition; fold pi/2 into cos (cos x = sin(x+pi/2))
    bias = np.empty((P, NCOMBO), np.float32)
    for mt in range(4):
        bias[:, mt] = b_cos[mt * P : (mt + 1) * P] + np.float32(np.pi / 2)
        bias[:, 4 + mt] = b_sin[mt * P : (mt + 1) * P]
    # DVE bias rows (plain b_cos, added in PSUM by K=1 matmul)
    biasd = b_cos.reshape(1, 4, P).astype(bf)
    # w_out.T with 1/31 mean folded in, chunked to [p, combo, o]
    wot = (w_out.T.astype(np.float64) / NWIN).astype(np.float32)
    wot = wot.reshape(NCOMBO, P, OUT_DIM).transpose(1, 0, 2).astype(bf)
    bot = np.broadcast_to(b_out.astype(np.float32), (P, OUT_DIM)).copy()

    in_maps = []
    for c in range(N_CORES):
        xs = x[c * B_LOCAL : (c + 1) * B_LOCAL, :]
        xt = np.ascontiguousarray(xs.T).astype(f8)  # [4096, 512]
        in_maps.append(
            {"x": xt, "wt": wt, "bias": bias, "biasd": biasd,
             "wot": wot, "bot": bot}
        )
    return in_maps


def run(inputs, trace=False, trace_cores=None):
    """Run the kernel; returns (y_full [4096,4] f32, BassKernelResults).

    Retries on transient device errors (the terminal occasionally reports
    NRT_EXEC_UNIT_UNRECOVERABLE after a prior crashed session and recovers
    on the next attempt)."""
    import time

    nc = _get_nc()
    in_maps = _make_in_maps(**inputs)
    last_err = None
    for attempt in range(3):
        try:
            res = run_bass_kernel_spmd(
                nc,
                in_maps,
                core_ids=list(range(N_CORES)),
                trace=trace,
                trace_cores=trace_cores,
            )
            y = np.concatenate([r["y"] for r in res.results], axis=0)
            return y, res
        except Exception as e:  # transient device wedge -> retry
            last_err = e
            if "UNRECOVERABLE" not in str(e) and "UNAVAILABLE" not in str(e):
                raise
            time.sleep(2.0)
    raise last_err


def kernel(**inputs):
    y, _ = run(inputs, trace=False)
    return y
